# revision 3
# baseline (speedup 1.0000x reference)
"""Trainium2 Bass kernel for GQA attention block with KV cache.

Computation (matches the reference):
    q = x @ Wq; k = x @ Wk; v = x @ Wv            (no bias)
    k, v = concat(past, new) along seq            (GQA: 8 kv heads, 32 q heads)
    out = softmax(q k^T / sqrt(hd) + mask) v
    out = out @ Wo

Sharding across 8 NeuronCores (one full TRN2 chip), done inside kernel():
  - Tensor-parallel over heads for projections + attention: core c owns
    q-heads 4c..4c+3 and kv head c (Wq/Wk/Wv column-sharded).
  - The attention output is exchanged with two AllToAlls (heads 0-1 fire
    halfway through the attention phase, heads 2-3 at the end) so that each
    core ends up with ALL heads for 1/8 of the tokens, then does the
    out-projection token-sharded against the full Wo.
  - All matmul operands are bf16 (fp32 accumulation in PSUM): full PE rate,
    fast-weight-load LDWEIGHTS, half the DMA/SBUF traffic.  Softmax scores
    are exponentiated two PSUM banks at a time (one ACT instruction per two
    key tiles) to amortize the ~300ns ACT instruction overhead.
  - Softmax denominators accumulate in a PSUM bank via ones-matmuls; the
    reciprocal uses the fast Newton-Raphson DVE approximation (~18 bits,
    plenty for bf16 data), and the normalization multiply reads the PSUM
    accumulator directly.

The mask input is handled exactly: host-side, exp(mask) is classified per
512x128 block into all-pass / all-blocked / partial; partial blocks are
shipped as multiplicative bf16 masks applied post-exp (0/1 values exact).
"""
import sys
import types
import numpy as np


def _ensure_axon_hooks():
    """antenv.axon_hooks may be absent; provide a stub so trace=True paths in
    bass_utils never crash on import.  Registers the real NTFF profiler hook
    when the boot helper is available (harmless otherwise)."""
    try:
        import antenv.axon_hooks  # noqa: F401
        return
    except Exception:
        pass
    mod = types.ModuleType("antenv.axon_hooks")
    mod._hook = None
    mod.set_axon_ntff_profile_hook = lambda h: setattr(mod, "_hook", h)
    mod.get_axon_ntff_profile_hook = lambda: mod._hook
    sys.modules["antenv.axon_hooks"] = mod
    try:
        import antenv
        antenv.axon_hooks = mod
    except Exception:
        pass
    try:
        from trn_agent_boot.trn_boot import _ntff_profile_via_ctypes
        mod._hook = _ntff_profile_via_ctypes("/opt/axon/libaxon_pjrt.so")
    except Exception:
        mod._hook = None


_ensure_axon_hooks()

import ml_dtypes
import concourse.bacc as bacc
import concourse.mybir as mybir
import concourse.tile as tile
from concourse import bass_utils
from concourse.masks import make_identity

F32 = mybir.dt.float32
F32R = mybir.dt.float32r
BF16 = mybir.dt.bfloat16
AF = mybir.ActivationFunctionType
NPBF = ml_dtypes.bfloat16

B, L, P, D = 2, 2048, 512, 4096
H, KH, HD = 32, 8, 128
S = P + L            # 2560 keys
W = 8                # cores
HPC = H // W         # 4 q heads per core
TQ = 512             # tokens per q-chunk (also the all-to-all chunk)
NQC = L // TQ        # 4 q chunks per sequence
NTOK = B * L         # 4096
NCH = NTOK // TQ     # 8 token chunks == W
ND = D // 128        # 32 contraction tiles for D
NS = S // 128        # 20 key tiles
GRP = 2              # score tiles per exp group (2 PSUM banks per ACT call)
ALPHA = 1.0 / float(HD) ** 0.5

LAST_RESULT = None   # BassKernelResults of the most recent run (for test harness)


def _mask_plan(mask):
    """mask: [L, S] additive attention mask (shared across batch/head).

    Returns (plan, mblocks): plan[qc] is a list of (key_tile, mblock_idx|None)
    to compute for queries [qc*TQ, (qc+1)*TQ); mblocks is [n, 128, TQ] float32,
    the exp(mask) of partial blocks transposed to [key, query] layout.
    exp is exact for the 0 / -1e9 masks (1.0 / 0.0)."""
    with np.errstate(over="ignore", under="ignore"):
        me = np.exp(mask.astype(np.float64)).astype(np.float32)
    plan = []
    blocks = []
    block_ids = {}
    for qc in range(NQC):
        row = []
        sub = me[qc * TQ:(qc + 1) * TQ]
        for st in range(NS):
            blk = sub[:, st * 128:(st + 1) * 128]
            if np.all(blk == 0.0):
                continue
            if np.all(blk == 1.0):
                row.append((st, None))
            else:
                bt = np.ascontiguousarray(blk.T)
                key = bt.tobytes()
                if key not in block_ids:
                    block_ids[key] = len(blocks)
                    blocks.append(bt)
                row.append((st, block_ids[key]))
        plan.append(row)
    mb = np.stack(blocks) if blocks else np.zeros((1, 128, TQ), np.float32)
    return plan, mb


def _build(plan, n_mb):
    nc = bacc.Bacc(None, target_bir_lowering=False, debug=False)
    xT = nc.dram_tensor("xT", [D, NTOK], BF16, kind="ExternalInput").ap()
    wq = nc.dram_tensor("wq", [D, HPC * HD], BF16, kind="ExternalInput").ap()
    wk = nc.dram_tensor("wk", [D, HD], BF16, kind="ExternalInput").ap()
    wv = nc.dram_tensor("wv", [D, HD], BF16, kind="ExternalInput").ap()
    pkT = nc.dram_tensor("pkT", [B, HD, P], BF16, kind="ExternalInput").ap()
    pv = nc.dram_tensor("pv", [B, P, HD], BF16, kind="ExternalInput").ap()
    wo = nc.dram_tensor("wo", [D, D], BF16, kind="ExternalInput").ap()
    mbk = nc.dram_tensor("mbk", [n_mb, 128, TQ], BF16, kind="ExternalInput").ap()
    out = nc.dram_tensor("out", [TQ, D], F32, kind="ExternalOutput").ap()

    # DRAM scratch
    qt_d = nc.dram_tensor("qt_d", [B, HPC, HD, L], BF16).ap()   # Q^T per head
    kt_d = nc.dram_tensor("kt_d", [B, HD, L], BF16).ap()        # new K^T
    v_d = nc.dram_tensor("v_d", [B, L, HD], BF16).ap()          # new V
    # all-to-all in two halves (heads 0-1 fire halfway through attention,
    # heads 2-3 at the end, hidden behind the out-projection start)
    a2a_in = [nc.dram_tensor("a2a_in0", [NCH, 2 * HD, TQ], BF16),
              nc.dram_tensor("a2a_in1", [NCH, 2 * HD, TQ], BF16)]
    a2a_out = [nc.dram_tensor("a2a_out0", [NCH, 2 * HD, TQ], BF16),
               nc.dram_tensor("a2a_out1", [NCH, 2 * HD, TQ], BF16)]

    from contextlib import ExitStack
    with tile.TileContext(nc) as tc, nc.allow_low_precision("bf16 matmul pipeline"):
        bstack = ExitStack()
        kvp = bstack.enter_context(tc.tile_pool(name="kvp", bufs=1))
        smb = bstack.enter_context(tc.tile_pool(name="smb", bufs=1))
        # Resident attention inputs (K^T, V, masks) allocated up front so their
        # loads overlap the projection phase instead of serializing behind it.
        ones_f = smb.tile([128, 1], F32, name="ones_f")
        nc.vector.memset(ones_f[:], 1.0)
        ones_s = smb.tile([128, 1], BF16, name="ones_s")
        nc.vector.tensor_copy(ones_s[:], ones_f[:])
        ones_1f = smb.tile([1, 128], F32, name="ones_1f")
        nc.vector.memset(ones_1f[:], 1.0)
        ones_1 = smb.tile([1, 128], F32R, name="ones_1")
        nc.vector.tensor_copy(ones_1[:], ones_1f[:])
        mb_t = smb.tile([128, n_mb, TQ], BF16, name="mb_t")
        ktbs, vbs = [], []
        for b in range(B):
            ktb = kvp.tile([128, S], BF16, name=f"ktb{b}", tag=f"ktb{b}")
            vb = kvp.tile([128, NS, 128], BF16, name=f"vb{b}", tag=f"vb{b}")
            ktbs.append(ktb)
            vbs.append(vb)

        def load_attn_prelude():
            # bulk mask/past-KV loads, deferred so they don't head-of-line
            # block the projection-critical DMAs at kernel start
            nc.sync.dma_start(out=mb_t[:], in_=mbk.rearrange("n p t -> p n t"))
            for b in range(B):
                nc.sync.dma_start(out=ktbs[b][:, 0:P], in_=pkT[b])
                nc.sync.dma_start(out=vbs[b][:, 0:P // 128, :],
                                  in_=pv[b].rearrange("(st p) d -> p st d", p=128))

        # ---------------- Phase A: Q/K/V projections ----------------
        with tc.tile_pool(name="wpool", bufs=1) as wp, \
             tc.tile_pool(name="xkp", bufs=2) as xkp, \
             tc.tile_pool(name="evp", bufs=4) as evp, \
             tc.tile_pool(name="cstA", bufs=1) as cstA, \
             tc.tile_pool(name="psA", bufs=7, space="PSUM") as psA, \
             tc.tile_pool(name="pstr", bufs=1, space="PSUM") as pstr:
            identAf = cstA.tile([128, 128], F32, name="identAf")
            make_identity(nc, identAf[:])
            identA = cstA.tile([128, 128], BF16, name="identA")
            nc.vector.tensor_copy(identA[:], identAf[:])
            wq_t = wp.tile([128, ND, HPC * HD], BF16, name="wq_t")
            wk_t = wp.tile([128, ND, HD], BF16, name="wk_t")
            wv_t = wp.tile([128, ND, HD], BF16, name="wv_t")
            wqr = wq.rearrange("(nd p) m -> p nd m", p=128)
            wkr = wk.rearrange("(nd p) m -> p nd m", p=128)
            wvr = wv.rearrange("(nd p) m -> p nd m", p=128)

            def load_weights(k0, k1):
                for k in range(k0, k1):
                    nc.sync.dma_start(out=wq_t[:, k, :], in_=wqr[:, k, :])
                    nc.sync.dma_start(out=wk_t[:, k, :], in_=wkr[:, k, :])
                    nc.sync.dma_start(out=wv_t[:, k, :], in_=wvr[:, k, :])

            xTr = xT.rearrange("(nd p) t -> p nd t", p=128)
            NQ4 = 4
            HF = ND // NQ4
            for tch in range(NCH):
                b, lc = tch // NQC, tch % NQC
                ps_list = [psA.tile([128, TQ], F32, name="psA_t", tag="psA_t")
                           for _ in range(6)]
                for hf in range(NQ4):
                    if tch == 0:
                        # feed the weight loads in lockstep with the first
                        # x chunk so the first matmul isn't stuck behind
                        # the full weight DMA
                        load_weights(hf * HF, (hf + 1) * HF)
                    xk = xkp.tile([128, HF, TQ], BF16, name="xk", tag="xk")
                    nc.sync.dma_start(
                        out=xk[:],
                        in_=xTr[:, hf * HF:(hf + 1) * HF, tch * TQ:(tch + 1) * TQ])
                    for o in range(6):
                        for kk in range(HF):
                            k = hf * HF + kk
                            if o < 4:
                                lhsT = wq_t[:, k, o * 128:(o + 1) * 128]
                            elif o == 4:
                                lhsT = wk_t[:, k, :]
                            else:
                                lhsT = wv_t[:, k, :]
                            nc.tensor.matmul(ps_list[o][:], lhsT=lhsT,
                                             rhs=xk[:, kk, :],
                                             start=(k == 0), stop=(k == ND - 1))
                for o in range(4):
                    ev = evp.tile([128, TQ], BF16, name="ev", tag="ev")
                    nc.vector.tensor_copy(ev[:], ps_list[o][:])
                    nc.sync.dma_start(out=qt_d[b, o, :, lc * TQ:(lc + 1) * TQ], in_=ev[:])
                evk = evp.tile([128, TQ], BF16, name="evk", tag="ev")
                nc.vector.tensor_copy(evk[:], ps_list[4][:])
                nc.sync.dma_start(out=kt_d[b, :, lc * TQ:(lc + 1) * TQ], in_=evk[:])
                # V comes out of the projection transposed [d, s]; flip to [s, d]
                evv = evp.tile([128, TQ], BF16, name="evv", tag="ev")
                nc.vector.tensor_copy(evv[:], ps_list[5][:])
                for i in range(4):
                    pt = pstr.tile([128, 128], BF16, name="pt", tag="pt")
                    nc.tensor.transpose(pt[:], evv[:, i * 128:(i + 1) * 128], identA[:])
                    ev2 = evp.tile([128, 128], BF16, name="ev2", tag="ev2")
                    nc.vector.tensor_copy(ev2[:], pt[:])
                    nc.sync.dma_start(
                        out=v_d[b, lc * TQ + i * 128: lc * TQ + (i + 1) * 128, :],
                        in_=ev2[:])
                if tch == 0:
                    load_attn_prelude()
                if lc == NQC - 1:
                    # this batch's K/V is complete; stage it for attention now
                    nc.sync.dma_start(out=ktbs[b][:, P:S], in_=kt_d[b])
                    nc.sync.dma_start(
                        out=vbs[b][:, P // 128:NS, :],
                        in_=v_d[b].rearrange("(st p) d -> p st d", p=128))

        # ---------------- Phase B: attention per (b, head, q-chunk) ----------------
        with tc.tile_pool(name="qtp", bufs=2) as qtp, \
             tc.tile_pool(name="esp", bufs=3) as esp, \
             tc.tile_pool(name="atp", bufs=2) as atp, \
             tc.tile_pool(name="psS", bufs=2, space="PSUM") as psS, \
             tc.tile_pool(name="psO", bufs=2, space="PSUM") as psO, \
             tc.tile_pool(name="psD", bufs=2, space="PSUM") as psD:
            pending = []  # deferred normalization of the previous chunk

            def flush_norm():
                if not pending:
                    return
                po, pd, b_, h_, qc_ = pending.pop()
                rd = atp.tile([1, TQ], F32, name="rd", tag="rd")
                nc.vector.reciprocal_approx_fast(rd[:], pd[:])
                rdc = atp.tile([1, TQ], F32R, name="rdc", tag="rdc")
                nc.vector.tensor_copy(rdc[:], rd[:])
                pb = psS.tile([128, GRP, TQ], F32, name="pb", tag="ps")
                nc.tensor.matmul(pb[:, 0, :], lhsT=ones_1[:], rhs=rdc[:])
                oev = atp.tile([128, TQ], BF16, name="oev", tag="oev")
                nc.vector.tensor_copy(oev[:], po[:])
                at = atp.tile([128, TQ], BF16, name="at", tag="at")
                nc.vector.tensor_mul(at[:], oev[:], pb[:, 0, :])
                half, hr = h_ // 2, h_ % 2
                nc.sync.dma_start(
                    out=a2a_in[half].ap()[b_ * NQC + qc_,
                                          hr * 128:(hr + 1) * 128, :],
                    in_=at[:])

            for h in range(HPC):
                for b in range(B):
                    ktb, vb = ktbs[b], vbs[b]
                    qt = qtp.tile([128, L], BF16, name="qt", tag="qt")
                    nc.sync.dma_start(out=qt[:], in_=qt_d[b, h])
                    for qc in range(NQC):
                        qtc = qt[:, qc * TQ:(qc + 1) * TQ]
                        po = psO.tile([128, TQ], F32, name="po", tag="po")
                        pd = psD.tile([1, TQ], F32, name="pd", tag="pd")
                        row = plan[qc]
                        nrow = len(row)
                        groups = [row[i:i + GRP] for i in range(0, nrow, GRP)]
                        idx = 0
                        for gi, grp in enumerate(groups):
                            ng = len(grp)
                            ps = psS.tile([128, GRP, TQ], F32, name="ps", tag="ps")
                            for j, (st, mb) in enumerate(grp):
                                nc.tensor.matmul(
                                    ps[:, j, :],
                                    lhsT=ktb[:, st * 128:(st + 1) * 128],
                                    rhs=qtc)
                            es = esp.tile([128, GRP, TQ], BF16, name="es", tag="es")
                            nc.scalar.activation(es[:, 0:ng, :], ps[:, 0:ng, :],
                                                 AF.Exp, scale=ALPHA)
                            for j, (st, mb) in enumerate(grp):
                                if mb is not None:
                                    nc.vector.tensor_mul(es[:, j, :], es[:, j, :],
                                                         mb_t[:, mb, :])
                            for j, (st, mb) in enumerate(grp):
                                first, last = (idx == 0), (idx == nrow - 1)
                                nc.tensor.matmul(po[:], lhsT=vb[:, st, :],
                                                 rhs=es[:, j, :],
                                                 start=first, stop=last)
                                nc.tensor.matmul(pd[:], lhsT=ones_s[:],
                                                 rhs=es[:, j, :],
                                                 start=first, stop=last)
                                idx += 1
                            if gi == 1:
                                flush_norm()  # previous chunk, now overlapped
                        pending.append((po, pd, b, h, qc))
                if h == 1:
                    flush_norm()
                    nc.gpsimd.collective_compute(
                        "AllToAll", mybir.AluOpType.bypass,
                        ins=[a2a_in[0].ap()], outs=[a2a_out[0].ap()],
                        replica_groups=[list(range(W))])
            flush_norm()
            nc.gpsimd.collective_compute(
                "AllToAll", mybir.AluOpType.bypass,
                ins=[a2a_in[1].ap()], outs=[a2a_out[1].ap()],
                replica_groups=[list(range(W))])

        bstack.close()  # release K/V/mask residency before the out-projection
        # ---------------- Phase C: out projection, token-sharded ----------------
        # Stationary operand = a 128x128 token tile of the gathered attention
        # output, reused across both 512-col matmuls of each wo row-tile.
        # 8 passes over 512 output columns each; within a pass the 32 head
        # tiles accumulate into 4 double-buffered PSUM banks (one per token
        # tile).  Head visit order puts the early all-to-all half first.
        with tc.tile_pool(name="a2ap", bufs=1) as a2ap, \
             tc.tile_pool(name="wop", bufs=6) as wop, \
             tc.tile_pool(name="evC", bufs=8) as evC, \
             tc.tile_pool(name="psC", bufs=2, space="PSUM") as psC:
            asb = a2ap.tile([128, H, TQ], BF16, name="asb")
            for half in range(2):
                for w in range(W):
                    for hh in range(2):
                        nc.sync.dma_start(
                            out=asb[:, w * HPC + half * 2 + hh, :],
                            in_=a2a_out[half].ap()[w, hh * 128:(hh + 1) * 128, :])
            # heads from a2a half 0 first, then half 1
            ht_order = [w * HPC + half * 2 + hh
                        for half in range(2) for w in range(W) for hh in range(2)]
            wor = wo.rearrange("(nh p) dd -> p nh dd", p=128)
            for dq in range(D // TQ):
                pc = psC.tile([128, NQC, TQ], F32, name="pc", tag="pc")
                for i, ht in enumerate(ht_order):
                    wt = wop.tile([128, TQ], BF16, name="wt", tag="wt")
                    nc.sync.dma_start(out=wt[:],
                                      in_=wor[:, ht, dq * TQ:(dq + 1) * TQ])
                    for tt in range(NQC):
                        nc.tensor.matmul(pc[:, tt, :],
                                         lhsT=asb[:, ht, tt * 128:(tt + 1) * 128],
                                         rhs=wt[:],
                                         start=(i == 0), stop=(i == H - 1))
                for tt in range(NQC):
                    evc = evC.tile([128, TQ], F32, name="evc", tag="evc")
                    nc.vector.tensor_copy(evc[:], pc[:, tt, :])
                    nc.sync.dma_start(
                        out=out[tt * 128:(tt + 1) * 128, dq * TQ:(dq + 1) * TQ],
                        in_=evc[:])

    nc.compile()
    return nc


def kernel(**inputs):
    global LAST_RESULT
    x = np.asarray(inputs["x"], np.float32)
    mask = np.asarray(inputs["mask"], np.float32)[0, 0]
    past_k = np.asarray(inputs["past_k"], np.float32)
    past_v = np.asarray(inputs["past_v"], np.float32)
    Wq = np.asarray(inputs["Wq"], np.float32)
    Wk = np.asarray(inputs["Wk"], np.float32)
    Wv = np.asarray(inputs["Wv"], np.float32)
    Wo = np.asarray(inputs["Wo"], np.float32)

    plan, mb = _mask_plan(mask)
    nc = _build(plan, mb.shape[0])

    xT = np.ascontiguousarray(x.reshape(NTOK, D).T.astype(NPBF))
    mbb = mb.astype(NPBF)
    wob = np.ascontiguousarray(Wo.astype(NPBF))
    in_maps = []
    for c in range(W):
        in_maps.append({
            "xT": xT,
            "wq": np.ascontiguousarray(
                Wq[:, c * HPC * HD:(c + 1) * HPC * HD].astype(NPBF)),
            "wk": np.ascontiguousarray(Wk[:, c * HD:(c + 1) * HD].astype(NPBF)),
            "wv": np.ascontiguousarray(Wv[:, c * HD:(c + 1) * HD].astype(NPBF)),
            "pkT": np.ascontiguousarray(
                past_k[:, c].transpose(0, 2, 1).astype(NPBF)),
            "pv": np.ascontiguousarray(past_v[:, c].astype(NPBF)),
            "wo": wob,
            "mbk": mbb,
        })
    res = None
    for attempt in range(3):
        try:
            res = bass_utils.run_bass_kernel_spmd(nc, in_maps, list(range(W)))
            break
        except Exception:
            if attempt == 2:
                raise
            import time as _time
            try:
                import jax as _jax
                _jax.clear_caches()
            except Exception:
                pass
            _time.sleep(3)
    LAST_RESULT = res
    out = np.empty((B, L, D), np.float32)
    for c in range(W):
        b, qc = c // NQC, c % NQC
        out[b, qc * TQ:(qc + 1) * TQ] = res.results[c]["out"]
    return out


# revision 7
# speedup vs baseline: 1.1822x; 1.1822x over previous
"""Trainium2 Bass kernel for GQA attention block with KV cache.

Computation (matches the reference):
    q = x @ Wq; k = x @ Wk; v = x @ Wv            (no bias)
    k, v = concat(past, new) along seq            (GQA: 8 kv heads, 32 q heads)
    out = softmax(q k^T / sqrt(hd) + mask) v
    out = out @ Wo

Sharding across 8 NeuronCores (one full TRN2 chip), done inside kernel():
  - Tensor-parallel over heads for projections + attention: core c owns
    q-heads 4c..4c+3 and kv head c (Wq/Wk/Wv column-sharded).
  - The attention output is exchanged with two AllToAlls (heads 0-1 fire
    halfway through the attention phase, heads 2-3 at the end) so that each
    core ends up with ALL heads for 1/8 of the tokens, then does the
    out-projection token-sharded against the full Wo.
  - All matmul operands are bf16 (fp32 accumulation in PSUM): full PE rate,
    fast-weight-load LDWEIGHTS, half the DMA/SBUF traffic.  Softmax scores
    are exponentiated two PSUM banks at a time (one ACT instruction per two
    key tiles) to amortize the ~300ns ACT instruction overhead.
  - Softmax denominators accumulate in a PSUM bank via ones-matmuls; the
    reciprocal uses the fast Newton-Raphson DVE approximation (~18 bits,
    plenty for bf16 data), and the normalization multiply reads the PSUM
    accumulator directly.

The mask input is handled exactly: host-side, exp(mask) is classified per
512x128 block into all-pass / all-blocked / partial; partial blocks are
shipped as multiplicative bf16 masks applied post-exp (0/1 values exact).
"""
import sys
import types
import numpy as np


def _ensure_axon_hooks():
    """antenv.axon_hooks may be absent; provide a stub so trace=True paths in
    bass_utils never crash on import.  Registers the real NTFF profiler hook
    when the boot helper is available (harmless otherwise)."""
    try:
        import antenv.axon_hooks  # noqa: F401
        return
    except Exception:
        pass
    mod = types.ModuleType("antenv.axon_hooks")
    mod._hook = None
    mod.set_axon_ntff_profile_hook = lambda h: setattr(mod, "_hook", h)
    mod.get_axon_ntff_profile_hook = lambda: mod._hook
    sys.modules["antenv.axon_hooks"] = mod
    try:
        import antenv
        antenv.axon_hooks = mod
    except Exception:
        pass
    try:
        from trn_agent_boot.trn_boot import _ntff_profile_via_ctypes
        mod._hook = _ntff_profile_via_ctypes("/opt/axon/libaxon_pjrt.so")
    except Exception:
        mod._hook = None


_ensure_axon_hooks()

import ml_dtypes
import concourse.bacc as bacc
import concourse.mybir as mybir
import concourse.tile as tile
from concourse import bass_utils
from concourse.masks import make_identity

F32 = mybir.dt.float32
F32R = mybir.dt.float32r
BF16 = mybir.dt.bfloat16
AF = mybir.ActivationFunctionType
NPBF = ml_dtypes.bfloat16

B, L, P, D = 2, 2048, 512, 4096
H, KH, HD = 32, 8, 128
S = P + L            # 2560 keys
W = 8                # cores
HPC = H // W         # 4 q heads per core
TQ = 512             # tokens per q-chunk (also the all-to-all chunk)
NQC = L // TQ        # 4 q chunks per sequence
NTOK = B * L         # 4096
NCH = NTOK // TQ     # 8 token chunks == W
ND = D // 128        # 32 contraction tiles for D
NS = S // 128        # 20 key tiles
GRP = 3              # score tiles per exp group (3 PSUM banks per ACT call)
ALPHA = 1.0 / float(HD) ** 0.5

LAST_RESULT = None   # BassKernelResults of the most recent run (for test harness)


def _mask_plan(mask):
    """mask: [L, S] additive attention mask (shared across batch/head).

    Returns (plan, mblocks): plan[qc] is a list of (key_tile, mblock_idx|None)
    to compute for queries [qc*TQ, (qc+1)*TQ); mblocks is [n, 128, TQ] float32,
    the exp(mask) of partial blocks transposed to [key, query] layout.
    exp is exact for the 0 / -1e9 masks (1.0 / 0.0)."""
    with np.errstate(over="ignore", under="ignore"):
        me = np.exp(mask.astype(np.float64)).astype(np.float32)
    plan = []
    blocks = []
    block_ids = {}
    for qc in range(NQC):
        row = []
        sub = me[qc * TQ:(qc + 1) * TQ]
        for st in range(NS):
            blk = sub[:, st * 128:(st + 1) * 128]
            if np.all(blk == 0.0):
                continue
            if np.all(blk == 1.0):
                row.append((st, None))
            else:
                bt = np.ascontiguousarray(blk.T)
                key = bt.tobytes()
                if key not in block_ids:
                    block_ids[key] = len(blocks)
                    blocks.append(bt)
                row.append((st, block_ids[key]))
        plan.append(row)
    mb = np.stack(blocks) if blocks else np.zeros((1, 128, TQ), np.float32)
    return plan, mb


def _build(plan, n_mb):
    nc = bacc.Bacc(None, target_bir_lowering=False, debug=False)
    xT = nc.dram_tensor("xT", [D, NTOK], BF16, kind="ExternalInput").ap()
    wq = nc.dram_tensor("wq", [D, HPC * HD], BF16, kind="ExternalInput").ap()
    wk = nc.dram_tensor("wk", [D, HD], BF16, kind="ExternalInput").ap()
    wv = nc.dram_tensor("wv", [D, HD], BF16, kind="ExternalInput").ap()
    pkT = nc.dram_tensor("pkT", [B, HD, P], BF16, kind="ExternalInput").ap()
    pv = nc.dram_tensor("pv", [B, P, HD], BF16, kind="ExternalInput").ap()
    wo = nc.dram_tensor("wo", [D, D], BF16, kind="ExternalInput").ap()
    mbk = nc.dram_tensor("mbk", [n_mb, 128, TQ], BF16, kind="ExternalInput").ap()
    out = nc.dram_tensor("out", [TQ, D], F32, kind="ExternalOutput").ap()

    # DRAM scratch
    qt_d = nc.dram_tensor("qt_d", [B, HPC, HD, L], BF16).ap()   # Q^T per head
    kt_d = nc.dram_tensor("kt_d", [B, HD, L], BF16).ap()        # new K^T
    v_d = nc.dram_tensor("v_d", [B, L, HD], BF16).ap()          # new V
    # all-to-all in four quarters (head h of every core fires as soon as that
    # head's chunks are flushed, so only the last quarter can expose latency)
    a2a_in = [nc.dram_tensor(f"a2a_in{h}", [NCH, HD, TQ], BF16)
              for h in range(HPC)]
    a2a_out = [nc.dram_tensor(f"a2a_out{h}", [NCH, HD, TQ], BF16)
               for h in range(HPC)]

    from contextlib import ExitStack
    with tile.TileContext(nc) as tc, nc.allow_low_precision("bf16 matmul pipeline"):
        bstack = ExitStack()
        kvp = bstack.enter_context(tc.tile_pool(name="kvp", bufs=1))
        smb = bstack.enter_context(tc.tile_pool(name="smb", bufs=1))
        # Resident attention inputs (K^T, V, masks) allocated up front so their
        # loads overlap the projection phase instead of serializing behind it.
        ones_f = smb.tile([128, 1], F32, name="ones_f")
        nc.vector.memset(ones_f[:], 1.0)
        ones_s = smb.tile([128, 1], BF16, name="ones_s")
        nc.vector.tensor_copy(ones_s[:], ones_f[:])
        ones_1f = smb.tile([1, 128], F32, name="ones_1f")
        nc.vector.memset(ones_1f[:], 1.0)
        ones_1 = smb.tile([1, 128], F32R, name="ones_1")
        nc.vector.tensor_copy(ones_1[:], ones_1f[:])
        mb_t = smb.tile([128, n_mb, TQ], BF16, name="mb_t")
        ktbs, vbs = [], []
        for b in range(B):
            ktb = kvp.tile([128, S], BF16, name=f"ktb{b}", tag=f"ktb{b}")
            vb = kvp.tile([128, NS, 128], BF16, name=f"vb{b}", tag=f"vb{b}")
            ktbs.append(ktb)
            vbs.append(vb)

        def load_attn_prelude():
            # bulk mask/past-KV loads, deferred so they don't head-of-line
            # block the projection-critical DMAs at kernel start
            nc.sync.dma_start(out=mb_t[:], in_=mbk.rearrange("n p t -> p n t"))
            for b in range(B):
                nc.sync.dma_start(out=ktbs[b][:, 0:P], in_=pkT[b])
                nc.sync.dma_start(out=vbs[b][:, 0:P // 128, :],
                                  in_=pv[b].rearrange("(st p) d -> p st d", p=128))

        # ---------------- Phase A: Q/K/V projections ----------------
        with tc.tile_pool(name="wpool", bufs=1) as wp, \
             tc.tile_pool(name="xkp", bufs=2) as xkp, \
             tc.tile_pool(name="evp", bufs=4) as evp, \
             tc.tile_pool(name="cstA", bufs=1) as cstA, \
             tc.tile_pool(name="psA", bufs=7, space="PSUM") as psA, \
             tc.tile_pool(name="pstr", bufs=1, space="PSUM") as pstr:
            identAf = cstA.tile([128, 128], F32, name="identAf")
            make_identity(nc, identAf[:])
            identA = cstA.tile([128, 128], BF16, name="identA")
            nc.vector.tensor_copy(identA[:], identAf[:])
            wq_t = wp.tile([128, ND, HPC * HD], BF16, name="wq_t")
            wk_t = wp.tile([128, ND, HD], BF16, name="wk_t")
            wv_t = wp.tile([128, ND, HD], BF16, name="wv_t")
            wqr = wq.rearrange("(nd p) m -> p nd m", p=128)
            wkr = wk.rearrange("(nd p) m -> p nd m", p=128)
            wvr = wv.rearrange("(nd p) m -> p nd m", p=128)

            def load_weights(k0, k1):
                for k in range(k0, k1):
                    nc.sync.dma_start(out=wq_t[:, k, :], in_=wqr[:, k, :])
                    nc.sync.dma_start(out=wk_t[:, k, :], in_=wkr[:, k, :])
                    nc.sync.dma_start(out=wv_t[:, k, :], in_=wvr[:, k, :])

            xTr = xT.rearrange("(nd p) t -> p nd t", p=128)
            NQ4 = 4
            HF = ND // NQ4
            for tch in range(NCH):
                b, lc = tch // NQC, tch % NQC
                ps_list = [psA.tile([128, TQ], F32, name="psA_t", tag="psA_t")
                           for _ in range(6)]
                for hf in range(NQ4):
                    if tch == 0:
                        # feed the weight loads in lockstep with the first
                        # x chunk so the first matmul isn't stuck behind
                        # the full weight DMA
                        load_weights(hf * HF, (hf + 1) * HF)
                    xk = xkp.tile([128, HF, TQ], BF16, name="xk", tag="xk")
                    nc.sync.dma_start(
                        out=xk[:],
                        in_=xTr[:, hf * HF:(hf + 1) * HF, tch * TQ:(tch + 1) * TQ])
                    for o in range(6):
                        for kk in range(HF):
                            k = hf * HF + kk
                            if o < 4:
                                lhsT = wq_t[:, k, o * 128:(o + 1) * 128]
                            elif o == 4:
                                lhsT = wk_t[:, k, :]
                            else:
                                lhsT = wv_t[:, k, :]
                            nc.tensor.matmul(ps_list[o][:], lhsT=lhsT,
                                             rhs=xk[:, kk, :],
                                             start=(k == 0), stop=(k == ND - 1))
                for o in range(4):
                    ev = evp.tile([128, TQ], BF16, name="ev", tag="ev")
                    nc.vector.tensor_copy(ev[:], ps_list[o][:])
                    nc.sync.dma_start(out=qt_d[b, o, :, lc * TQ:(lc + 1) * TQ], in_=ev[:])
                evk = evp.tile([128, TQ], BF16, name="evk", tag="ev")
                nc.vector.tensor_copy(evk[:], ps_list[4][:])
                nc.sync.dma_start(out=kt_d[b, :, lc * TQ:(lc + 1) * TQ], in_=evk[:])
                # V comes out of the projection transposed [d, s]; flip to [s, d]
                evv = evp.tile([128, TQ], BF16, name="evv", tag="ev")
                nc.vector.tensor_copy(evv[:], ps_list[5][:])
                for i in range(4):
                    pt = pstr.tile([128, 128], BF16, name="pt", tag="pt")
                    nc.tensor.transpose(pt[:], evv[:, i * 128:(i + 1) * 128], identA[:])
                    ev2 = evp.tile([128, 128], BF16, name="ev2", tag="ev2")
                    nc.vector.tensor_copy(ev2[:], pt[:])
                    nc.sync.dma_start(
                        out=v_d[b, lc * TQ + i * 128: lc * TQ + (i + 1) * 128, :],
                        in_=ev2[:])
                if tch == 0:
                    load_attn_prelude()
                if lc == NQC - 1:
                    # this batch's K/V is complete; stage it for attention now
                    nc.sync.dma_start(out=ktbs[b][:, P:S], in_=kt_d[b])
                    nc.sync.dma_start(
                        out=vbs[b][:, P // 128:NS, :],
                        in_=v_d[b].rearrange("(st p) d -> p st d", p=128))

        # ---------------- Phase B: attention per (b, head, q-chunk) ----------------
        # Scores are computed GRP key-tiles at a time into one multi-bank PSUM
        # tile, exponentiated with a single ACT instruction, and the AV
        # matmuls run one group BEHIND the scores so the in-order PE queue
        # never head-of-line blocks on the exp.  Softmax denominators
        # accumulate on the DVE (bf16) and are reduced by one ones-matmul per
        # chunk at flush time; the per-lane bf16 rounding averages out in the
        # 128-partition fp32 PSUM reduction.
        with tc.tile_pool(name="qtp", bufs=2) as qtp, \
             tc.tile_pool(name="esp", bufs=3) as esp, \
             tc.tile_pool(name="atp", bufs=2) as atp, \
             tc.tile_pool(name="dap", bufs=2) as dap, \
             tc.tile_pool(name="psS", bufs=2, space="PSUM") as psS, \
             tc.tile_pool(name="psO", bufs=2, space="PSUM") as psO:
            pending = []  # deferred normalization of the previous chunk

            def flush_norm():
                if not pending:
                    return
                po, dacc, b_, h_, qc_ = pending.pop(0)
                pp = psS.tile([128, GRP, TQ], F32, name="pp", tag="ps")
                nc.tensor.matmul(pp[0:1, 0, :], lhsT=ones_s[:], rhs=dacc[:])
                rd = atp.tile([1, TQ], F32, name="rd", tag="rd")
                nc.vector.reciprocal_approx_fast(rd[:], pp[0:1, 0, :])
                rdc = atp.tile([1, TQ], F32R, name="rdc", tag="rdc")
                nc.vector.tensor_copy(rdc[:], rd[:])
                nc.tensor.matmul(pp[:, 1, :], lhsT=ones_1[:], rhs=rdc[:])
                oev = atp.tile([128, TQ], BF16, name="oev", tag="oev")
                nc.vector.tensor_copy(oev[:], po[:])
                at = atp.tile([128, TQ], BF16, name="at", tag="at")
                nc.vector.tensor_mul(at[:], oev[:], pp[:, 1, :])
                nc.sync.dma_start(
                    out=a2a_in[h_].ap()[b_ * NQC + qc_, :, :], in_=at[:])

            for h in range(HPC):
                for b in range(B):
                    ktb, vb = ktbs[b], vbs[b]
                    qt = qtp.tile([128, L], BF16, name="qt", tag="qt")
                    nc.sync.dma_start(out=qt[:], in_=qt_d[b, h])
                    for qc in range(NQC):
                        qtc = qt[:, qc * TQ:(qc + 1) * TQ]
                        po = psO.tile([128, TQ], F32, name="po", tag="po")
                        dacc = dap.tile([128, TQ], BF16, name="dacc", tag="dacc")
                        row = plan[qc]
                        nrow = len(row)
                        groups = [row[i:i + GRP] for i in range(0, nrow, GRP)]
                        idx = 0

                        def issue_av(prev):
                            nonlocal idx
                            pgrp, pes = prev
                            for j, (st, mb) in enumerate(pgrp):
                                nc.tensor.matmul(po[:], lhsT=vb[:, st, :],
                                                 rhs=pes[:, j, :],
                                                 start=(idx == 0),
                                                 stop=(idx == nrow - 1))
                                idx += 1

                        prev = None
                        for gi, grp in enumerate(groups):
                            ng = len(grp)
                            ps = psS.tile([128, GRP, TQ], F32, name="ps", tag="ps")
                            for j, (st, mb) in enumerate(grp):
                                nc.tensor.matmul(
                                    ps[:, j, :],
                                    lhsT=ktb[:, st * 128:(st + 1) * 128],
                                    rhs=qtc)
                            if prev is not None:
                                issue_av(prev)
                            es = esp.tile([128, GRP, TQ], BF16, name="es", tag="es")
                            nc.scalar.activation(es[:, 0:ng, :], ps[:, 0:ng, :],
                                                 AF.Exp, scale=ALPHA)
                            for j, (st, mb) in enumerate(grp):
                                if mb is not None:
                                    nc.vector.tensor_mul(es[:, j, :], es[:, j, :],
                                                         mb_t[:, mb, :])
                            base_t = gi * GRP
                            for j in range(ng):
                                if base_t + j == 0:
                                    nc.vector.tensor_copy(dacc[:], es[:, j, :])
                                else:
                                    nc.vector.tensor_add(dacc[:], dacc[:],
                                                         es[:, j, :])
                            if gi == 1:
                                flush_norm()  # previous chunk, now overlapped
                            prev = (grp, es)
                        issue_av(prev)
                        pending.append((po, dacc, b, h, qc))
                # drain this head's chunks and ship them to their token owners
                while pending:
                    flush_norm()
                nc.gpsimd.collective_compute(
                    "AllToAll", mybir.AluOpType.bypass,
                    ins=[a2a_in[h].ap()], outs=[a2a_out[h].ap()],
                    replica_groups=[list(range(W))])

        bstack.close()  # release K/V/mask residency before the out-projection
        # ---------------- Phase C: out projection, token-sharded ----------------
        # Stationary operand = a 128x128 token tile of the gathered attention
        # output, reused across both 512-col matmuls of each wo row-tile.
        # 8 passes over 512 output columns each; within a pass the 32 head
        # tiles accumulate into 4 double-buffered PSUM banks (one per token
        # tile).  Head visit order puts the early all-to-all half first.
        with tc.tile_pool(name="a2ap", bufs=1) as a2ap, \
             tc.tile_pool(name="wop", bufs=6) as wop, \
             tc.tile_pool(name="evC", bufs=8) as evC, \
             tc.tile_pool(name="psC", bufs=2, space="PSUM") as psC:
            asb = a2ap.tile([128, H, TQ], BF16, name="asb")
            for hq in range(HPC):
                for w in range(W):
                    nc.sync.dma_start(out=asb[:, w * HPC + hq, :],
                                      in_=a2a_out[hq].ap()[w, :, :])
            # heads from the earliest all-to-all quarters first, so only the
            # last quarter can stall the accumulation
            ht_order = [w * HPC + hq for hq in range(HPC) for w in range(W)]
            wor = wo.rearrange("(nh p) dd -> p nh dd", p=128)
            for dq in range(D // TQ):
                pc = psC.tile([128, NQC, TQ], F32, name="pc", tag="pc")
                for i, ht in enumerate(ht_order):
                    wt = wop.tile([128, TQ], BF16, name="wt", tag="wt")
                    nc.sync.dma_start(out=wt[:],
                                      in_=wor[:, ht, dq * TQ:(dq + 1) * TQ])
                    for tt in range(NQC):
                        nc.tensor.matmul(pc[:, tt, :],
                                         lhsT=asb[:, ht, tt * 128:(tt + 1) * 128],
                                         rhs=wt[:],
                                         start=(i == 0), stop=(i == H - 1))
                for tt in range(NQC):
                    evc = evC.tile([128, TQ], F32, name="evc", tag="evc")
                    nc.vector.tensor_copy(evc[:], pc[:, tt, :])
                    nc.sync.dma_start(
                        out=out[tt * 128:(tt + 1) * 128, dq * TQ:(dq + 1) * TQ],
                        in_=evc[:])

    nc.compile()
    return nc


def kernel(**inputs):
    global LAST_RESULT
    x = np.asarray(inputs["x"], np.float32)
    mask = np.asarray(inputs["mask"], np.float32)[0, 0]
    past_k = np.asarray(inputs["past_k"], np.float32)
    past_v = np.asarray(inputs["past_v"], np.float32)
    Wq = np.asarray(inputs["Wq"], np.float32)
    Wk = np.asarray(inputs["Wk"], np.float32)
    Wv = np.asarray(inputs["Wv"], np.float32)
    Wo = np.asarray(inputs["Wo"], np.float32)

    plan, mb = _mask_plan(mask)
    nc = _build(plan, mb.shape[0])

    xT = np.ascontiguousarray(x.reshape(NTOK, D).T.astype(NPBF))
    mbb = mb.astype(NPBF)
    wob = np.ascontiguousarray(Wo.astype(NPBF))
    in_maps = []
    for c in range(W):
        in_maps.append({
            "xT": xT,
            "wq": np.ascontiguousarray(
                Wq[:, c * HPC * HD:(c + 1) * HPC * HD].astype(NPBF)),
            "wk": np.ascontiguousarray(Wk[:, c * HD:(c + 1) * HD].astype(NPBF)),
            "wv": np.ascontiguousarray(Wv[:, c * HD:(c + 1) * HD].astype(NPBF)),
            "pkT": np.ascontiguousarray(
                past_k[:, c].transpose(0, 2, 1).astype(NPBF)),
            "pv": np.ascontiguousarray(past_v[:, c].astype(NPBF)),
            "wo": wob,
            "mbk": mbb,
        })
    res = None
    for attempt in range(3):
        try:
            res = bass_utils.run_bass_kernel_spmd(nc, in_maps, list(range(W)))
            break
        except Exception:
            if attempt == 2:
                raise
            import time as _time
            try:
                import jax as _jax
                _jax.clear_caches()
            except Exception:
                pass
            _time.sleep(3)
    LAST_RESULT = res
    out = np.empty((B, L, D), np.float32)
    for c in range(W):
        b, qc = c // NQC, c % NQC
        out[b, qc * TQ:(qc + 1) * TQ] = res.results[c]["out"]
    return out


# revision 15
# speedup vs baseline: 1.2202x; 1.0321x over previous
"""Trainium2 Bass kernel for GQA attention block with KV cache.

Computation (matches the reference):
    q = x @ Wq; k = x @ Wk; v = x @ Wv            (no bias)
    k, v = concat(past, new) along seq            (GQA: 8 kv heads, 32 q heads)
    out = softmax(q k^T / sqrt(hd) + mask) v
    out = out @ Wo

Sharding across 8 NeuronCores (one full TRN2 chip), done inside kernel():
  - Tensor-parallel over heads for projections + attention: core c owns
    q-heads 4c..4c+3 and kv head c (Wq/Wk/Wv column-sharded).
  - The attention output is exchanged with two AllToAlls (heads 0-1 fire
    halfway through the attention phase, heads 2-3 at the end) so that each
    core ends up with ALL heads for 1/8 of the tokens, then does the
    out-projection token-sharded against the full Wo.
  - All matmul operands are bf16 (fp32 accumulation in PSUM): full PE rate,
    fast-weight-load LDWEIGHTS, half the DMA/SBUF traffic.  Softmax scores
    are exponentiated two PSUM banks at a time (one ACT instruction per two
    key tiles) to amortize the ~300ns ACT instruction overhead.
  - Softmax denominators accumulate in a PSUM bank via ones-matmuls; the
    reciprocal uses the fast Newton-Raphson DVE approximation (~18 bits,
    plenty for bf16 data), and the normalization multiply reads the PSUM
    accumulator directly.

The mask input is handled exactly: host-side, exp(mask) is classified per
512x128 block into all-pass / all-blocked / partial; partial blocks are
shipped as multiplicative bf16 masks applied post-exp (0/1 values exact).
"""
import sys
import types
import numpy as np


def _ensure_axon_hooks():
    """antenv.axon_hooks may be absent; provide a stub so trace=True paths in
    bass_utils never crash on import.  Registers the real NTFF profiler hook
    when the boot helper is available (harmless otherwise)."""
    try:
        import antenv.axon_hooks  # noqa: F401
        return
    except Exception:
        pass
    mod = types.ModuleType("antenv.axon_hooks")
    mod._hook = None
    mod.set_axon_ntff_profile_hook = lambda h: setattr(mod, "_hook", h)
    mod.get_axon_ntff_profile_hook = lambda: mod._hook
    sys.modules["antenv.axon_hooks"] = mod
    try:
        import antenv
        antenv.axon_hooks = mod
    except Exception:
        pass
    try:
        from trn_agent_boot.trn_boot import _ntff_profile_via_ctypes
        mod._hook = _ntff_profile_via_ctypes("/opt/axon/libaxon_pjrt.so")
    except Exception:
        mod._hook = None


_ensure_axon_hooks()

import ml_dtypes
import concourse.bacc as bacc
import concourse.mybir as mybir
import concourse.tile as tile
from concourse import bass_utils
from concourse.masks import make_identity

F32 = mybir.dt.float32
F32R = mybir.dt.float32r
BF16 = mybir.dt.bfloat16
AF = mybir.ActivationFunctionType
NPBF = ml_dtypes.bfloat16

B, L, P, D = 2, 2048, 512, 4096
H, KH, HD = 32, 8, 128
S = P + L            # 2560 keys
W = 8                # cores
HPC = H // W         # 4 q heads per core
TQ = 512             # tokens per q-chunk (also the all-to-all chunk)
NQC = L // TQ        # 4 q chunks per sequence
NTOK = B * L         # 4096
NCH = NTOK // TQ     # 8 token chunks == W
ND = D // 128        # 32 contraction tiles for D
NS = S // 128        # 20 key tiles
GRP = 3              # score tiles per exp group (3 PSUM banks per ACT call)
ALPHA = 1.0 / float(HD) ** 0.5

LAST_RESULT = None   # BassKernelResults of the most recent run (for test harness)


def _mask_plan(mask):
    """mask: [L, S] additive attention mask (shared across batch/head).

    Returns (plan, mblocks): plan[qc] is a list of (key_tile, mblock_idx|None)
    to compute for queries [qc*TQ, (qc+1)*TQ); mblocks is [n, 128, TQ] float32,
    the exp(mask) of partial blocks transposed to [key, query] layout.
    exp is exact for the 0 / -1e9 masks (1.0 / 0.0)."""
    with np.errstate(over="ignore", under="ignore"):
        me = np.exp(mask.astype(np.float64)).astype(np.float32)
    plan = []
    blocks = []
    block_ids = {}
    for qc in range(NQC):
        row = []
        sub = me[qc * TQ:(qc + 1) * TQ]
        for st in range(NS):
            blk = sub[:, st * 128:(st + 1) * 128]
            if np.all(blk == 0.0):
                continue
            if np.all(blk == 1.0):
                row.append((st, None))
            else:
                bt = np.ascontiguousarray(blk.T)
                key = bt.tobytes()
                if key not in block_ids:
                    block_ids[key] = len(blocks)
                    blocks.append(bt)
                row.append((st, block_ids[key]))
        plan.append(row)
    mb = np.stack(blocks) if blocks else np.zeros((1, 128, TQ), np.float32)
    return plan, mb


def _build(plan, n_mb):
    nc = bacc.Bacc(None, target_bir_lowering=False, debug=False)
    xT = nc.dram_tensor("xT", [D, NTOK], BF16, kind="ExternalInput").ap()
    wq = nc.dram_tensor("wq", [D, HPC * HD], BF16, kind="ExternalInput").ap()
    wk = nc.dram_tensor("wk", [D, HD], BF16, kind="ExternalInput").ap()
    wv = nc.dram_tensor("wv", [D, HD], BF16, kind="ExternalInput").ap()
    pkT = nc.dram_tensor("pkT", [B, HD, P], BF16, kind="ExternalInput").ap()
    pv = nc.dram_tensor("pv", [B, P, HD], BF16, kind="ExternalInput").ap()
    wo = nc.dram_tensor("wo", [D, D], BF16, kind="ExternalInput").ap()
    mbk = nc.dram_tensor("mbk", [n_mb, 128, TQ], BF16, kind="ExternalInput").ap()
    out = nc.dram_tensor("out", [TQ, D], F32, kind="ExternalOutput").ap()

    # DRAM scratch
    qt_d = nc.dram_tensor("qt_d", [B, HPC, HD, L], BF16).ap()   # Q^T per head
    kt_d = nc.dram_tensor("kt_d", [B, HD, L], BF16).ap()        # new K^T
    v_d = nc.dram_tensor("v_d", [B, L, HD], BF16).ap()          # new V
    # all-to-all in four quarters (head h of every core fires as soon as that
    # head's chunks are flushed, so only the last quarter can expose latency)
    a2a_in = [nc.dram_tensor(f"a2a_in{h}", [NCH, HD, TQ], BF16)
              for h in range(HPC)]
    a2a_out = [nc.dram_tensor(f"a2a_out{h}", [NCH, HD, TQ], BF16)
               for h in range(HPC)]

    from contextlib import ExitStack
    with tile.TileContext(nc) as tc, nc.allow_low_precision("bf16 matmul pipeline"):
        cstack = ExitStack()
        a2ap = cstack.enter_context(tc.tile_pool(name="a2ap", bufs=1))
        wop = cstack.enter_context(tc.tile_pool(name="wop", bufs=10))
        bstack = ExitStack()
        kvp = bstack.enter_context(tc.tile_pool(name="kvp", bufs=1))
        smb = bstack.enter_context(tc.tile_pool(name="smb", bufs=1))
        # Resident attention inputs (K^T, V, masks) allocated up front so their
        # loads overlap the projection phase instead of serializing behind it.
        ones_f = smb.tile([128, 1], F32, name="ones_f")
        nc.vector.memset(ones_f[:], 1.0)
        ones_s = smb.tile([128, 1], BF16, name="ones_s")
        nc.vector.tensor_copy(ones_s[:], ones_f[:])
        ones_1f = smb.tile([1, 128], F32, name="ones_1f")
        nc.vector.memset(ones_1f[:], 1.0)
        ones_1 = smb.tile([1, 128], F32R, name="ones_1")
        nc.vector.tensor_copy(ones_1[:], ones_1f[:])
        mb_t = smb.tile([128, n_mb, TQ], BF16, name="mb_t")
        ktbs, vbs = [], []
        for b in range(B):
            ktb = kvp.tile([128, S], BF16, name=f"ktb{b}", tag=f"ktb{b}")
            vb = kvp.tile([128, NS, 128], BF16, name=f"vb{b}", tag=f"vb{b}")
            ktbs.append(ktb)
            vbs.append(vb)

        def load_attn_prelude():
            # bulk mask/past-KV loads, deferred so they don't head-of-line
            # block the projection-critical DMAs at kernel start
            nc.sync.dma_start(out=mb_t[:], in_=mbk.rearrange("n p t -> p n t"))
            for b in range(B):
                nc.sync.dma_start(out=ktbs[b][:, 0:P], in_=pkT[b])
                nc.sync.dma_start(out=vbs[b][:, 0:P // 128, :],
                                  in_=pv[b].rearrange("(st p) d -> p st d", p=128))

        # ---------------- Phase A: Q/K/V projections ----------------
        with tc.tile_pool(name="wpool", bufs=1) as wp, \
             tc.tile_pool(name="xkp", bufs=2) as xkp, \
             tc.tile_pool(name="evp", bufs=4) as evp, \
             tc.tile_pool(name="cstA", bufs=1) as cstA, \
             tc.tile_pool(name="psA", bufs=7, space="PSUM") as psA, \
             tc.tile_pool(name="pstr", bufs=1, space="PSUM") as pstr:
            identAf = cstA.tile([128, 128], F32, name="identAf")
            make_identity(nc, identAf[:])
            identA = cstA.tile([128, 128], BF16, name="identA")
            nc.vector.tensor_copy(identA[:], identAf[:])
            wq_t = wp.tile([128, ND, HPC * HD], BF16, name="wq_t")
            wk_t = wp.tile([128, ND, HD], BF16, name="wk_t")
            wv_t = wp.tile([128, ND, HD], BF16, name="wv_t")
            wqr = wq.rearrange("(nd p) m -> p nd m", p=128)
            wkr = wk.rearrange("(nd p) m -> p nd m", p=128)
            wvr = wv.rearrange("(nd p) m -> p nd m", p=128)

            def load_weights(k0, k1):
                for k in range(k0, k1):
                    nc.sync.dma_start(out=wq_t[:, k, :], in_=wqr[:, k, :])
                    nc.sync.dma_start(out=wk_t[:, k, :], in_=wkr[:, k, :])
                    nc.sync.dma_start(out=wv_t[:, k, :], in_=wvr[:, k, :])

            xTr = xT.rearrange("(nd p) t -> p nd t", p=128)
            NQ4 = 4
            HF = ND // NQ4
            for tch in range(NCH):
                b, lc = tch // NQC, tch % NQC
                ps_list = [psA.tile([128, TQ], F32, name="psA_t", tag="psA_t")
                           for _ in range(6)]
                for hf in range(NQ4):
                    if tch == 0:
                        # feed the weight loads in lockstep with the first
                        # x chunk so the first matmul isn't stuck behind
                        # the full weight DMA
                        load_weights(hf * HF, (hf + 1) * HF)
                    xk = xkp.tile([128, HF, TQ], BF16, name="xk", tag="xk")
                    nc.sync.dma_start(
                        out=xk[:],
                        in_=xTr[:, hf * HF:(hf + 1) * HF, tch * TQ:(tch + 1) * TQ])
                    for o in range(6):
                        for kk in range(HF):
                            k = hf * HF + kk
                            if o < 4:
                                lhsT = wq_t[:, k, o * 128:(o + 1) * 128]
                            elif o == 4:
                                lhsT = wk_t[:, k, :]
                            else:
                                lhsT = wv_t[:, k, :]
                            nc.tensor.matmul(ps_list[o][:], lhsT=lhsT,
                                             rhs=xk[:, kk, :],
                                             start=(k == 0), stop=(k == ND - 1))
                for o in range(4):
                    ev = evp.tile([128, TQ], BF16, name="ev", tag="ev")
                    nc.vector.tensor_copy(ev[:], ps_list[o][:])
                    nc.sync.dma_start(out=qt_d[b, o, :, lc * TQ:(lc + 1) * TQ], in_=ev[:])
                evk = evp.tile([128, TQ], BF16, name="evk", tag="ev")
                nc.vector.tensor_copy(evk[:], ps_list[4][:])
                nc.sync.dma_start(out=kt_d[b, :, lc * TQ:(lc + 1) * TQ], in_=evk[:])
                # V comes out of the projection transposed [d, s]; flip to [s, d]
                evv = evp.tile([128, TQ], BF16, name="evv", tag="ev")
                nc.vector.tensor_copy(evv[:], ps_list[5][:])
                for i in range(4):
                    pt = pstr.tile([128, 128], BF16, name="pt", tag="pt")
                    nc.tensor.transpose(pt[:], evv[:, i * 128:(i + 1) * 128], identA[:])
                    ev2 = evp.tile([128, 128], BF16, name="ev2", tag="ev2")
                    nc.vector.tensor_copy(ev2[:], pt[:])
                    nc.sync.dma_start(
                        out=v_d[b, lc * TQ + i * 128: lc * TQ + (i + 1) * 128, :],
                        in_=ev2[:])
                if tch == 0:
                    load_attn_prelude()
                if lc == NQC - 1:
                    # this batch's K/V is complete; stage it for attention now
                    nc.sync.dma_start(out=ktbs[b][:, P:S], in_=kt_d[b])
                    nc.sync.dma_start(
                        out=vbs[b][:, P // 128:NS, :],
                        in_=v_d[b].rearrange("(st p) d -> p st d", p=128))

        # Phase C staging (pools opened above, before the B-phase pools, to
        # keep pool open/close in stack order): the gathered attention output
        # and the first out-projection weight tiles stream in during the
        # attention phase instead of stalling the out-projection start.
        asb = a2ap.tile([128, H, TQ], BF16, name="asb")
        wor = wo.rearrange("(nh p) dd -> p nh dd", p=128)
        # heads from the earliest all-to-all quarters first, so only the
        # last quarter can stall the out-projection accumulation
        ht_order = [w * HPC + hq for hq in range(HPC) for w in range(W)]
        wt_sched = [(dq, ht) for dq in range(D // TQ) for ht in ht_order]
        wt_ring = []

        def wt_prefetch():
            if not wt_sched:
                return
            dq_, ht_ = wt_sched.pop(0)
            wt = wop.tile([128, TQ], BF16, name="wt", tag="wt")
            nc.sync.dma_start(out=wt[:], in_=wor[:, ht_, dq_ * TQ:(dq_ + 1) * TQ])
            wt_ring.append(wt)

        # ---------------- Phase B: attention per (b, head, q-chunk) ----------------
        # Scores are computed GRP key-tiles at a time into one multi-bank PSUM
        # tile, exponentiated with a single ACT instruction, and the AV
        # matmuls run one group BEHIND the scores so the in-order PE queue
        # never head-of-line blocks on the exp.  Softmax denominators
        # accumulate on the DVE (bf16) and are reduced by one ones-matmul per
        # chunk at flush time; the per-lane bf16 rounding averages out in the
        # 128-partition fp32 PSUM reduction.
        with tc.tile_pool(name="qtp", bufs=2) as qtp, \
             tc.tile_pool(name="esp", bufs=3) as esp, \
             tc.tile_pool(name="atp", bufs=2) as atp, \
             tc.tile_pool(name="dap", bufs=2) as dap, \
             tc.tile_pool(name="psS", bufs=2, space="PSUM") as psS, \
             tc.tile_pool(name="psO", bufs=2, space="PSUM") as psO:
            pending = []  # deferred normalization of the previous chunk

            def flush_norm():
                if not pending:
                    return
                po, dacc3_, ncols, b_, h_, qc_ = pending.pop(0)
                if ncols > 1:
                    nc.vector.tensor_add(dacc3_[:, 0, :], dacc3_[:, 0, :],
                                         dacc3_[:, 1, :])
                if ncols > 2:
                    nc.vector.tensor_add(dacc3_[:, 0, :], dacc3_[:, 0, :],
                                         dacc3_[:, 2, :])
                pp = psS.tile([128, GRP, TQ], F32, name="pp", tag="ps")
                nc.tensor.matmul(pp[0:1, 0, :], lhsT=ones_s[:],
                                 rhs=dacc3_[:, 0, :])
                rd = atp.tile([1, TQ], F32, name="rd", tag="rd")
                nc.vector.reciprocal_approx_fast(rd[:], pp[0:1, 0, :])
                rdc = atp.tile([1, TQ], F32R, name="rdc", tag="rdc")
                nc.vector.tensor_copy(rdc[:], rd[:])
                nc.tensor.matmul(pp[:, 1, :], lhsT=ones_1[:], rhs=rdc[:])
                oev = atp.tile([128, TQ], BF16, name="oev", tag="oev")
                nc.vector.tensor_copy(oev[:], po[:])
                at = atp.tile([128, TQ], BF16, name="at", tag="at")
                nc.vector.tensor_mul(at[:], oev[:], pp[:, 1, :])
                nc.sync.dma_start(
                    out=a2a_in[h_].ap()[b_ * NQC + qc_, :, :], in_=at[:])

            for h in range(HPC):
                for b in range(B):
                    ktb, vb = ktbs[b], vbs[b]
                    qt = qtp.tile([128, L], BF16, name="qt", tag="qt")
                    nc.sync.dma_start(out=qt[:], in_=qt_d[b, h])
                    for qc in range(NQC):
                        qtc = qt[:, qc * TQ:(qc + 1) * TQ]
                        po = psO.tile([128, TQ], F32, name="po", tag="po")
                        dacc3 = dap.tile([128, GRP, TQ], BF16, name="dacc3",
                                         tag="dacc")
                        row = plan[qc]
                        nrow = len(row)
                        nlast = min(nrow, GRP)
                        groups = [row[i:i + GRP] for i in range(0, nrow, GRP)]
                        idx = 0

                        def issue_av(prev):
                            nonlocal idx
                            pgrp, pes = prev
                            for j, (st, mb) in enumerate(pgrp):
                                nc.tensor.matmul(po[:], lhsT=vb[:, st, :],
                                                 rhs=pes[:, j, :],
                                                 start=(idx == 0),
                                                 stop=(idx == nrow - 1))
                                idx += 1

                        prev = None
                        for gi, grp in enumerate(groups):
                            ng = len(grp)
                            ps = psS.tile([128, GRP, TQ], F32, name="ps", tag="ps")
                            for j, (st, mb) in enumerate(grp):
                                nc.tensor.matmul(
                                    ps[:, j, :],
                                    lhsT=ktb[:, st * 128:(st + 1) * 128],
                                    rhs=qtc)
                            if prev is not None:
                                issue_av(prev)
                            es = esp.tile([128, GRP, TQ], BF16, name="es", tag="es")
                            nc.scalar.activation(es[:, 0:ng, :], ps[:, 0:ng, :],
                                                 AF.Exp, scale=ALPHA)
                            # apply partial-mask blocks; consecutive tiles use
                            # consecutive mask slots, so one multiply covers
                            # every masked tile in the group
                            mrun = [(j, mb) for j, (st, mb) in enumerate(grp)
                                    if mb is not None]
                            if mrun:
                                j0, m0 = mrun[0]
                                k = len(mrun)
                                if all(mrun[i] == (j0 + i, m0 + i)
                                       for i in range(k)):
                                    nc.vector.tensor_mul(
                                        es[:, j0:j0 + k, :], es[:, j0:j0 + k, :],
                                        mb_t[:, m0:m0 + k, :])
                                else:
                                    for j, mb in mrun:
                                        nc.vector.tensor_mul(
                                            es[:, j, :], es[:, j, :],
                                            mb_t[:, mb, :])
                            # denominator: per-group wide accumulate (bf16 DVE
                            # runs 2x on wide 16-bit ops), folded at flush
                            if gi == 0:
                                nc.vector.tensor_copy(dacc3[:, 0:ng, :],
                                                      es[:, 0:ng, :])
                            else:
                                nc.vector.tensor_add(dacc3[:, 0:ng, :],
                                                     dacc3[:, 0:ng, :],
                                                     es[:, 0:ng, :])
                            if gi == 1:
                                flush_norm()  # previous chunk, now overlapped
                            prev = (grp, es)
                        issue_av(prev)
                        pending.append((po, dacc3, nlast, b, h, qc))
                # drain this head's chunks and ship them to their token owners
                while pending:
                    flush_norm()
                nc.gpsimd.collective_compute(
                    "AllToAll", mybir.AluOpType.bypass,
                    ins=[a2a_in[h].ap()], outs=[a2a_out[h].ap()],
                    replica_groups=[list(range(W))])
                # stage this quarter's gathered heads while attention continues
                for w in range(W):
                    nc.sync.dma_start(out=asb[:, w * HPC + h, :],
                                      in_=a2a_out[h].ap()[w, :, :])
            for _ in range(8):
                wt_prefetch()  # out-projection weights start streaming now

        bstack.close()  # release K/V/mask residency before the out-projection
        # ---------------- Phase C: out projection, token-sharded ----------------
        # Stationary operand = a 128x128 token tile of the gathered attention
        # output.  8 passes over 512 output columns each; within a pass the 32
        # head tiles accumulate into 4 double-buffered PSUM banks (one per
        # token tile).  Weight tiles stream through a software-pipelined
        # prefetch ring so a pass never waits on the DMA queue.
        with tc.tile_pool(name="evC", bufs=8) as evC, \
             tc.tile_pool(name="psC", bufs=2, space="PSUM") as psC:
            for dq in range(D // TQ):
                pc = psC.tile([128, NQC, TQ], F32, name="pc", tag="pc")
                for i, ht in enumerate(ht_order):
                    wt = wt_ring.pop(0)
                    wt_prefetch()
                    for tt in range(NQC):
                        nc.tensor.matmul(pc[:, tt, :],
                                         lhsT=asb[:, ht, tt * 128:(tt + 1) * 128],
                                         rhs=wt[:],
                                         start=(i == 0), stop=(i == H - 1))
                for tt in range(NQC):
                    evc = evC.tile([128, TQ], F32, name="evc", tag="evc")
                    nc.vector.tensor_copy(evc[:], pc[:, tt, :])
                    nc.sync.dma_start(
                        out=out[tt * 128:(tt + 1) * 128, dq * TQ:(dq + 1) * TQ],
                        in_=evc[:])
        cstack.close()

    nc.compile()
    return nc


def kernel(**inputs):
    global LAST_RESULT
    x = np.asarray(inputs["x"], np.float32)
    mask = np.asarray(inputs["mask"], np.float32)[0, 0]
    past_k = np.asarray(inputs["past_k"], np.float32)
    past_v = np.asarray(inputs["past_v"], np.float32)
    Wq = np.asarray(inputs["Wq"], np.float32)
    Wk = np.asarray(inputs["Wk"], np.float32)
    Wv = np.asarray(inputs["Wv"], np.float32)
    Wo = np.asarray(inputs["Wo"], np.float32)

    plan, mb = _mask_plan(mask)
    nc = _build(plan, mb.shape[0])

    xT = np.ascontiguousarray(x.reshape(NTOK, D).T.astype(NPBF))
    mbb = mb.astype(NPBF)
    wob = np.ascontiguousarray(Wo.astype(NPBF))
    in_maps = []
    for c in range(W):
        in_maps.append({
            "xT": xT,
            "wq": np.ascontiguousarray(
                Wq[:, c * HPC * HD:(c + 1) * HPC * HD].astype(NPBF)),
            "wk": np.ascontiguousarray(Wk[:, c * HD:(c + 1) * HD].astype(NPBF)),
            "wv": np.ascontiguousarray(Wv[:, c * HD:(c + 1) * HD].astype(NPBF)),
            "pkT": np.ascontiguousarray(
                past_k[:, c].transpose(0, 2, 1).astype(NPBF)),
            "pv": np.ascontiguousarray(past_v[:, c].astype(NPBF)),
            "wo": wob,
            "mbk": mbb,
        })
    res = None
    for attempt in range(3):
        try:
            res = bass_utils.run_bass_kernel_spmd(nc, in_maps, list(range(W)))
            break
        except Exception:
            if attempt == 2:
                raise
            import time as _time
            try:
                import jax as _jax
                _jax.clear_caches()
            except Exception:
                pass
            _time.sleep(3)
    LAST_RESULT = res
    out = np.empty((B, L, D), np.float32)
    for c in range(W):
        b, qc = c // NQC, c % NQC
        out[b, qc * TQ:(qc + 1) * TQ] = res.results[c]["out"]
    return out


# revision 16
# speedup vs baseline: 1.2230x; 1.0023x over previous
"""Trainium2 Bass kernel for GQA attention block with KV cache.

Computation (matches the reference):
    q = x @ Wq; k = x @ Wk; v = x @ Wv            (no bias)
    k, v = concat(past, new) along seq            (GQA: 8 kv heads, 32 q heads)
    out = softmax(q k^T / sqrt(hd) + mask) v
    out = out @ Wo

Sharding across 8 NeuronCores (one full TRN2 chip), done inside kernel():
  - Tensor-parallel over heads for projections + attention: core c owns
    q-heads 4c..4c+3 and kv head c (Wq/Wk/Wv column-sharded).
  - The attention output is exchanged with two AllToAlls (heads 0-1 fire
    halfway through the attention phase, heads 2-3 at the end) so that each
    core ends up with ALL heads for 1/8 of the tokens, then does the
    out-projection token-sharded against the full Wo.
  - All matmul operands are bf16 (fp32 accumulation in PSUM): full PE rate,
    fast-weight-load LDWEIGHTS, half the DMA/SBUF traffic.  Softmax scores
    are exponentiated two PSUM banks at a time (one ACT instruction per two
    key tiles) to amortize the ~300ns ACT instruction overhead.
  - Softmax denominators accumulate in a PSUM bank via ones-matmuls; the
    reciprocal uses the fast Newton-Raphson DVE approximation (~18 bits,
    plenty for bf16 data), and the normalization multiply reads the PSUM
    accumulator directly.

The mask input is handled exactly: host-side, exp(mask) is classified per
512x128 block into all-pass / all-blocked / partial; partial blocks are
shipped as multiplicative bf16 masks applied post-exp (0/1 values exact).
"""
import sys
import types
import numpy as np


def _ensure_axon_hooks():
    """antenv.axon_hooks may be absent; provide a stub so trace=True paths in
    bass_utils never crash on import.  Registers the real NTFF profiler hook
    when the boot helper is available (harmless otherwise)."""
    try:
        import antenv.axon_hooks  # noqa: F401
        return
    except Exception:
        pass
    mod = types.ModuleType("antenv.axon_hooks")
    mod._hook = None
    mod.set_axon_ntff_profile_hook = lambda h: setattr(mod, "_hook", h)
    mod.get_axon_ntff_profile_hook = lambda: mod._hook
    sys.modules["antenv.axon_hooks"] = mod
    try:
        import antenv
        antenv.axon_hooks = mod
    except Exception:
        pass
    try:
        from trn_agent_boot.trn_boot import _ntff_profile_via_ctypes
        mod._hook = _ntff_profile_via_ctypes("/opt/axon/libaxon_pjrt.so")
    except Exception:
        mod._hook = None


_ensure_axon_hooks()

import ml_dtypes
import concourse.bacc as bacc
import concourse.mybir as mybir
import concourse.tile as tile
from concourse import bass_utils
from concourse.masks import make_identity

F32 = mybir.dt.float32
F32R = mybir.dt.float32r
BF16 = mybir.dt.bfloat16
AF = mybir.ActivationFunctionType
NPBF = ml_dtypes.bfloat16

B, L, P, D = 2, 2048, 512, 4096
H, KH, HD = 32, 8, 128
S = P + L            # 2560 keys
W = 8                # cores
HPC = H // W         # 4 q heads per core
TQ = 512             # tokens per q-chunk (also the all-to-all chunk)
NQC = L // TQ        # 4 q chunks per sequence
NTOK = B * L         # 4096
NCH = NTOK // TQ     # 8 token chunks == W
ND = D // 128        # 32 contraction tiles for D
NS = S // 128        # 20 key tiles
GRP = 3              # score tiles per exp group (3 PSUM banks per ACT call)
ALPHA = 1.0 / float(HD) ** 0.5

LAST_RESULT = None   # BassKernelResults of the most recent run (for test harness)


def _mask_plan(mask):
    """mask: [L, S] additive attention mask (shared across batch/head).

    Returns (plan, mblocks): plan[qc] is a list of (key_tile, mblock_idx|None)
    to compute for queries [qc*TQ, (qc+1)*TQ); mblocks is [n, 128, TQ] float32,
    the exp(mask) of partial blocks transposed to [key, query] layout.
    exp is exact for the 0 / -1e9 masks (1.0 / 0.0)."""
    with np.errstate(over="ignore", under="ignore"):
        me = np.exp(mask.astype(np.float64)).astype(np.float32)
    plan = []
    blocks = []
    block_ids = {}
    for qc in range(NQC):
        row = []
        sub = me[qc * TQ:(qc + 1) * TQ]
        for st in range(NS):
            blk = sub[:, st * 128:(st + 1) * 128]
            if np.all(blk == 0.0):
                continue
            if np.all(blk == 1.0):
                row.append((st, None))
            else:
                bt = np.ascontiguousarray(blk.T)
                key = bt.tobytes()
                if key not in block_ids:
                    block_ids[key] = len(blocks)
                    blocks.append(bt)
                row.append((st, block_ids[key]))
        plan.append(row)
    mb = np.stack(blocks) if blocks else np.zeros((1, 128, TQ), np.float32)
    return plan, mb


def _build(plan, n_mb):
    nc = bacc.Bacc(None, target_bir_lowering=False, debug=False)
    xT = nc.dram_tensor("xT", [D, NTOK], BF16, kind="ExternalInput").ap()
    wq = nc.dram_tensor("wq", [D, HPC * HD], BF16, kind="ExternalInput").ap()
    wk = nc.dram_tensor("wk", [D, HD], BF16, kind="ExternalInput").ap()
    wv = nc.dram_tensor("wv", [D, HD], BF16, kind="ExternalInput").ap()
    pkT = nc.dram_tensor("pkT", [B, HD, P], BF16, kind="ExternalInput").ap()
    pv = nc.dram_tensor("pv", [B, P, HD], BF16, kind="ExternalInput").ap()
    wo = nc.dram_tensor("wo", [D, D], BF16, kind="ExternalInput").ap()
    mbk = nc.dram_tensor("mbk", [n_mb, 128, TQ], BF16, kind="ExternalInput").ap()
    out = nc.dram_tensor("out", [TQ, D], F32, kind="ExternalOutput").ap()

    # DRAM scratch
    qt_d = nc.dram_tensor("qt_d", [B, HPC, HD, L], BF16).ap()   # Q^T per head
    kt_d = nc.dram_tensor("kt_d", [B, HD, L], BF16).ap()        # new K^T
    v_d = nc.dram_tensor("v_d", [B, L, HD], BF16).ap()          # new V
    # all-to-all in four quarters (head h of every core fires as soon as that
    # head's chunks are flushed, so only the last quarter can expose latency)
    a2a_in = [nc.dram_tensor(f"a2a_in{h}", [NCH, HD, TQ], BF16)
              for h in range(HPC)]
    a2a_out = [nc.dram_tensor(f"a2a_out{h}", [NCH, HD, TQ], BF16)
               for h in range(HPC)]

    from contextlib import ExitStack
    with tile.TileContext(nc) as tc, nc.allow_low_precision("bf16 matmul pipeline"):
        cstack = ExitStack()
        a2ap = cstack.enter_context(tc.tile_pool(name="a2ap", bufs=1))
        wop = cstack.enter_context(tc.tile_pool(name="wop", bufs=10))
        bstack = ExitStack()
        kvp = bstack.enter_context(tc.tile_pool(name="kvp", bufs=1))
        smb = bstack.enter_context(tc.tile_pool(name="smb", bufs=1))
        # Resident attention inputs (K^T, V, masks) allocated up front so their
        # loads overlap the projection phase instead of serializing behind it.
        ones_f = smb.tile([128, 1], F32, name="ones_f")
        nc.vector.memset(ones_f[:], 1.0)
        ones_s = smb.tile([128, 1], BF16, name="ones_s")
        nc.vector.tensor_copy(ones_s[:], ones_f[:])
        ones_1f = smb.tile([1, 128], F32, name="ones_1f")
        nc.vector.memset(ones_1f[:], 1.0)
        ones_1 = smb.tile([1, 128], F32R, name="ones_1")
        nc.vector.tensor_copy(ones_1[:], ones_1f[:])
        mb_t = smb.tile([128, n_mb, TQ], BF16, name="mb_t")
        ktbs, vbs = [], []
        for b in range(B):
            ktb = kvp.tile([128, S], BF16, name=f"ktb{b}", tag=f"ktb{b}")
            vb = kvp.tile([128, NS, 128], BF16, name=f"vb{b}", tag=f"vb{b}")
            ktbs.append(ktb)
            vbs.append(vb)

        def load_attn_prelude():
            # bulk mask/past-KV loads, deferred so they don't head-of-line
            # block the projection-critical DMAs at kernel start
            nc.sync.dma_start(out=mb_t[:], in_=mbk.rearrange("n p t -> p n t"))
            for b in range(B):
                nc.sync.dma_start(out=ktbs[b][:, 0:P], in_=pkT[b])
                nc.sync.dma_start(out=vbs[b][:, 0:P // 128, :],
                                  in_=pv[b].rearrange("(st p) d -> p st d", p=128))

        # ---------------- Phase A: Q/K/V projections ----------------
        with tc.tile_pool(name="wpool", bufs=1) as wp, \
             tc.tile_pool(name="xkp", bufs=2) as xkp, \
             tc.tile_pool(name="evp", bufs=4) as evp, \
             tc.tile_pool(name="cstA", bufs=1) as cstA, \
             tc.tile_pool(name="psA", bufs=7, space="PSUM") as psA, \
             tc.tile_pool(name="pstr", bufs=1, space="PSUM") as pstr:
            identAf = cstA.tile([128, 128], F32, name="identAf")
            make_identity(nc, identAf[:])
            identA = cstA.tile([128, 128], BF16, name="identA")
            nc.vector.tensor_copy(identA[:], identAf[:])
            wq_t = wp.tile([128, ND, HPC * HD], BF16, name="wq_t")
            wk_t = wp.tile([128, ND, HD], BF16, name="wk_t")
            wv_t = wp.tile([128, ND, HD], BF16, name="wv_t")
            wqr = wq.rearrange("(nd p) m -> p nd m", p=128)
            wkr = wk.rearrange("(nd p) m -> p nd m", p=128)
            wvr = wv.rearrange("(nd p) m -> p nd m", p=128)

            def load_weights(k0, k1):
                for k in range(k0, k1):
                    nc.sync.dma_start(out=wq_t[:, k, :], in_=wqr[:, k, :])
                    nc.sync.dma_start(out=wk_t[:, k, :], in_=wkr[:, k, :])
                    nc.sync.dma_start(out=wv_t[:, k, :], in_=wvr[:, k, :])

            xTr = xT.rearrange("(nd p) t -> p nd t", p=128)
            NQ4 = 4
            HF = ND // NQ4
            for tch in range(NCH):
                b, lc = tch // NQC, tch % NQC
                ps_list = [psA.tile([128, TQ], F32, name="psA_t", tag="psA_t")
                           for _ in range(6)]
                for hf in range(NQ4):
                    if tch == 0:
                        # feed the weight loads in lockstep with the first
                        # x chunk so the first matmul isn't stuck behind
                        # the full weight DMA
                        load_weights(hf * HF, (hf + 1) * HF)
                    xk = xkp.tile([128, HF, TQ], BF16, name="xk", tag="xk")
                    nc.sync.dma_start(
                        out=xk[:],
                        in_=xTr[:, hf * HF:(hf + 1) * HF, tch * TQ:(tch + 1) * TQ])
                    for o in range(6):
                        for kk in range(HF):
                            k = hf * HF + kk
                            if o < 4:
                                lhsT = wq_t[:, k, o * 128:(o + 1) * 128]
                            elif o == 4:
                                lhsT = wk_t[:, k, :]
                            else:
                                lhsT = wv_t[:, k, :]
                            nc.tensor.matmul(ps_list[o][:], lhsT=lhsT,
                                             rhs=xk[:, kk, :],
                                             start=(k == 0), stop=(k == ND - 1))
                for o in range(4):
                    ev = evp.tile([128, TQ], BF16, name="ev", tag="ev")
                    nc.vector.tensor_copy(ev[:], ps_list[o][:])
                    nc.sync.dma_start(out=qt_d[b, o, :, lc * TQ:(lc + 1) * TQ], in_=ev[:])
                evk = evp.tile([128, TQ], BF16, name="evk", tag="ev")
                nc.vector.tensor_copy(evk[:], ps_list[4][:])
                nc.sync.dma_start(out=kt_d[b, :, lc * TQ:(lc + 1) * TQ], in_=evk[:])
                # V comes out of the projection transposed [d, s]; flip to [s, d]
                evv = evp.tile([128, TQ], BF16, name="evv", tag="ev")
                nc.vector.tensor_copy(evv[:], ps_list[5][:])
                for i in range(4):
                    pt = pstr.tile([128, 128], BF16, name="pt", tag="pt")
                    nc.tensor.transpose(pt[:], evv[:, i * 128:(i + 1) * 128], identA[:])
                    ev2 = evp.tile([128, 128], BF16, name="ev2", tag="ev2")
                    nc.vector.tensor_copy(ev2[:], pt[:])
                    nc.sync.dma_start(
                        out=v_d[b, lc * TQ + i * 128: lc * TQ + (i + 1) * 128, :],
                        in_=ev2[:])
                if tch == 0:
                    load_attn_prelude()
                if lc == NQC - 1:
                    # this batch's K/V is complete; stage it for attention now
                    nc.sync.dma_start(out=ktbs[b][:, P:S], in_=kt_d[b])
                    nc.sync.dma_start(
                        out=vbs[b][:, P // 128:NS, :],
                        in_=v_d[b].rearrange("(st p) d -> p st d", p=128))

        # Phase C staging (pools opened above, before the B-phase pools, to
        # keep pool open/close in stack order): the gathered attention output
        # and the first out-projection weight tiles stream in during the
        # attention phase instead of stalling the out-projection start.
        asb = a2ap.tile([128, H, TQ], BF16, name="asb")
        wor = wo.rearrange("(nh p) dd -> p nh dd", p=128)
        # heads from the earliest all-to-all quarters first, so only the
        # last quarter can stall the out-projection accumulation
        ht_order = [w * HPC + hq for hq in range(HPC) for w in range(W)]
        wt_sched = [(dq, ht) for dq in range(D // TQ) for ht in ht_order]
        wt_ring = []

        def wt_prefetch():
            if not wt_sched:
                return
            dq_, ht_ = wt_sched.pop(0)
            wt = wop.tile([128, TQ], BF16, name="wt", tag="wt")
            nc.sync.dma_start(out=wt[:], in_=wor[:, ht_, dq_ * TQ:(dq_ + 1) * TQ])
            wt_ring.append(wt)

        # ---------------- Phase B: attention per (b, head, q-chunk) ----------------
        # Scores are computed GRP key-tiles at a time into one multi-bank PSUM
        # tile, exponentiated with a single ACT instruction, and the AV
        # matmuls run one group BEHIND the scores so the in-order PE queue
        # never head-of-line blocks on the exp.  Softmax denominators
        # accumulate on the DVE (bf16) and are reduced by one ones-matmul per
        # chunk at flush time; the per-lane bf16 rounding averages out in the
        # 128-partition fp32 PSUM reduction.
        with tc.tile_pool(name="qtp", bufs=2) as qtp, \
             tc.tile_pool(name="esp", bufs=3) as esp, \
             tc.tile_pool(name="atp", bufs=2) as atp, \
             tc.tile_pool(name="dap", bufs=2) as dap, \
             tc.tile_pool(name="psS", bufs=2, space="PSUM") as psS, \
             tc.tile_pool(name="psO", bufs=2, space="PSUM") as psO:
            pending = []  # deferred normalization of the previous chunk

            def flush_norm():
                if not pending:
                    return
                po, dacc3_, ncols, b_, h_, qc_ = pending.pop(0)
                if ncols > 1:
                    nc.vector.tensor_add(dacc3_[:, 0, :], dacc3_[:, 0, :],
                                         dacc3_[:, 1, :])
                if ncols > 2:
                    nc.vector.tensor_add(dacc3_[:, 0, :], dacc3_[:, 0, :],
                                         dacc3_[:, 2, :])
                pp = psS.tile([128, GRP, TQ], F32, name="pp", tag="ps")
                nc.tensor.matmul(pp[0:1, 0, :], lhsT=ones_s[:],
                                 rhs=dacc3_[:, 0, :])
                rd = atp.tile([1, TQ], F32, name="rd", tag="rd")
                nc.vector.reciprocal_approx_fast(rd[:], pp[0:1, 0, :])
                rdc = atp.tile([1, TQ], F32R, name="rdc", tag="rdc")
                nc.vector.tensor_copy(rdc[:], rd[:])
                nc.tensor.matmul(pp[:, 1, :], lhsT=ones_1[:], rhs=rdc[:])
                oev = atp.tile([128, TQ], BF16, name="oev", tag="oev")
                nc.vector.tensor_copy(oev[:], po[:])
                at = atp.tile([128, TQ], BF16, name="at", tag="at")
                nc.vector.tensor_mul(at[:], oev[:], pp[:, 1, :])
                nc.sync.dma_start(
                    out=a2a_in[h_].ap()[b_ * NQC + qc_, :, :], in_=at[:])

            for h in range(HPC):
                for b in range(B):
                    ktb, vb = ktbs[b], vbs[b]
                    qt = qtp.tile([128, L], BF16, name="qt", tag="qt")
                    nc.sync.dma_start(out=qt[:], in_=qt_d[b, h])
                    for qc in range(NQC):
                        qtc = qt[:, qc * TQ:(qc + 1) * TQ]
                        po = psO.tile([128, TQ], F32, name="po", tag="po")
                        dacc3 = dap.tile([128, GRP, TQ], BF16, name="dacc3",
                                         tag="dacc")
                        row = plan[qc]
                        nrow = len(row)
                        nlast = min(nrow, GRP)
                        groups = [row[i:i + GRP] for i in range(0, nrow, GRP)]
                        idx = 0

                        def issue_av(prev):
                            nonlocal idx
                            pgrp, pes = prev
                            for j, (st, mb) in enumerate(pgrp):
                                nc.tensor.matmul(po[:], lhsT=vb[:, st, :],
                                                 rhs=pes[:, j, :],
                                                 start=(idx == 0),
                                                 stop=(idx == nrow - 1))
                                idx += 1

                        prev = None
                        for gi, grp in enumerate(groups):
                            ng = len(grp)
                            ps = psS.tile([128, GRP, TQ], F32, name="ps", tag="ps")
                            for j, (st, mb) in enumerate(grp):
                                nc.tensor.matmul(
                                    ps[:, j, :],
                                    lhsT=ktb[:, st * 128:(st + 1) * 128],
                                    rhs=qtc)
                            if prev is not None:
                                issue_av(prev)
                            es = esp.tile([128, GRP, TQ], BF16, name="es", tag="es")
                            nc.scalar.activation(es[:, 0:ng, :], ps[:, 0:ng, :],
                                                 AF.Exp, scale=ALPHA)
                            # apply partial-mask blocks; consecutive tiles use
                            # consecutive mask slots, so one multiply covers
                            # every masked tile in the group
                            mrun = [(j, mb) for j, (st, mb) in enumerate(grp)
                                    if mb is not None]
                            if mrun:
                                j0, m0 = mrun[0]
                                k = len(mrun)
                                if all(mrun[i] == (j0 + i, m0 + i)
                                       for i in range(k)):
                                    nc.vector.tensor_mul(
                                        es[:, j0:j0 + k, :], es[:, j0:j0 + k, :],
                                        mb_t[:, m0:m0 + k, :])
                                else:
                                    for j, mb in mrun:
                                        nc.vector.tensor_mul(
                                            es[:, j, :], es[:, j, :],
                                            mb_t[:, mb, :])
                            # denominator: per-group wide accumulate (bf16 DVE
                            # runs 2x on wide 16-bit ops), folded at flush
                            if gi == 0:
                                nc.vector.tensor_copy(dacc3[:, 0:ng, :],
                                                      es[:, 0:ng, :])
                            else:
                                nc.vector.tensor_add(dacc3[:, 0:ng, :],
                                                     dacc3[:, 0:ng, :],
                                                     es[:, 0:ng, :])
                            if gi == 1:
                                flush_norm()  # previous chunk, now overlapped
                            prev = (grp, es)
                        issue_av(prev)
                        pending.append((po, dacc3, nlast, b, h, qc))
                # drain this head's chunks and ship them to their token owners
                while pending:
                    flush_norm()
                nc.gpsimd.collective_compute(
                    "AllToAll", mybir.AluOpType.bypass,
                    ins=[a2a_in[h].ap()], outs=[a2a_out[h].ap()],
                    replica_groups=[list(range(W))])
            for _ in range(8):
                wt_prefetch()  # out-projection weights start streaming now
            # stage the gathered heads in accumulation order, after every
            # collective has completed (reading a quarter too eagerly races
            # the peers' remote writes)
            for hq in range(HPC):
                for w in range(W):
                    nc.sync.dma_start(out=asb[:, w * HPC + hq, :],
                                      in_=a2a_out[hq].ap()[w, :, :])

        bstack.close()  # release K/V/mask residency before the out-projection
        # ---------------- Phase C: out projection, token-sharded ----------------
        # Stationary operand = a 128x128 token tile of the gathered attention
        # output.  8 passes over 512 output columns each; within a pass the 32
        # head tiles accumulate into 4 double-buffered PSUM banks (one per
        # token tile).  Weight tiles stream through a software-pipelined
        # prefetch ring so a pass never waits on the DMA queue.
        with tc.tile_pool(name="evC", bufs=8) as evC, \
             tc.tile_pool(name="psC", bufs=2, space="PSUM") as psC:
            for dq in range(D // TQ):
                pc = psC.tile([128, NQC, TQ], F32, name="pc", tag="pc")
                for i, ht in enumerate(ht_order):
                    wt = wt_ring.pop(0)
                    wt_prefetch()
                    for tt in range(NQC):
                        nc.tensor.matmul(pc[:, tt, :],
                                         lhsT=asb[:, ht, tt * 128:(tt + 1) * 128],
                                         rhs=wt[:],
                                         start=(i == 0), stop=(i == H - 1))
                for tt in range(NQC):
                    evc = evC.tile([128, TQ], F32, name="evc", tag="evc")
                    nc.vector.tensor_copy(evc[:], pc[:, tt, :])
                    nc.sync.dma_start(
                        out=out[tt * 128:(tt + 1) * 128, dq * TQ:(dq + 1) * TQ],
                        in_=evc[:])
        cstack.close()

    nc.compile()
    return nc


def kernel(**inputs):
    global LAST_RESULT
    x = np.asarray(inputs["x"], np.float32)
    mask = np.asarray(inputs["mask"], np.float32)[0, 0]
    past_k = np.asarray(inputs["past_k"], np.float32)
    past_v = np.asarray(inputs["past_v"], np.float32)
    Wq = np.asarray(inputs["Wq"], np.float32)
    Wk = np.asarray(inputs["Wk"], np.float32)
    Wv = np.asarray(inputs["Wv"], np.float32)
    Wo = np.asarray(inputs["Wo"], np.float32)

    plan, mb = _mask_plan(mask)
    nc = _build(plan, mb.shape[0])

    xT = np.ascontiguousarray(x.reshape(NTOK, D).T.astype(NPBF))
    mbb = mb.astype(NPBF)
    wob = np.ascontiguousarray(Wo.astype(NPBF))
    in_maps = []
    for c in range(W):
        in_maps.append({
            "xT": xT,
            "wq": np.ascontiguousarray(
                Wq[:, c * HPC * HD:(c + 1) * HPC * HD].astype(NPBF)),
            "wk": np.ascontiguousarray(Wk[:, c * HD:(c + 1) * HD].astype(NPBF)),
            "wv": np.ascontiguousarray(Wv[:, c * HD:(c + 1) * HD].astype(NPBF)),
            "pkT": np.ascontiguousarray(
                past_k[:, c].transpose(0, 2, 1).astype(NPBF)),
            "pv": np.ascontiguousarray(past_v[:, c].astype(NPBF)),
            "wo": wob,
            "mbk": mbb,
        })
    res = None
    for attempt in range(3):
        try:
            res = bass_utils.run_bass_kernel_spmd(nc, in_maps, list(range(W)))
            break
        except Exception:
            if attempt == 2:
                raise
            import time as _time
            try:
                import jax as _jax
                _jax.clear_caches()
            except Exception:
                pass
            _time.sleep(3)
    LAST_RESULT = res
    out = np.empty((B, L, D), np.float32)
    for c in range(W):
        b, qc = c // NQC, c % NQC
        out[b, qc * TQ:(qc + 1) * TQ] = res.results[c]["out"]
    return out
